# revision 8
# baseline (speedup 1.0000x reference)
import os
import sys

import numpy as np

for _p in ("/opt/trn_rl_repo", "/root/.axon_site/_ro/trn_rl_repo"):
    if os.path.isdir(_p) and _p not in sys.path:
        sys.path.insert(0, _p)

# nn_CRF: feats [B,S,T] f32, masks [B,S] ones, transitions [T,T].
#
# Renorm-free exp-domain recursion X_{s+1} = exp(feat_{s+1}-F) * (E X_s).
# Products of positive matrices contract to rank-1 (Birkhoff), so the
# recursion forgets its start in ~10 steps; we run NQ=24 overlapping
# time-chunks per core IN PARALLEL (chunk 0 from the true X_0, chunk i from
# an arbitrary positive start ~14 steps before its range) and glue scales
# host-side with one per-batch scalar ratio per handoff.  Chunks are packed
# 8-wide into the free dim so each chain-iteration is ONE [128,128]x[128,256]
# matmul + ONE [128,256] DVE multiply; 3 such chains hide the per-step
# MM->sem->TT->sem round-trip (~650ns) while the DVE stays saturated.
B, S, T = 512, 1024, 64
NC = 8            # cores
BL = B // NC      # 64 batches per core
NQ = 24           # time-chunks per core
MW = 8            # chunks merged per chain
NCHAIN = NQ // MW  # 3
L_IT = 48         # iterations per chain (1 init copy + 47 steps)
# chunk i ends (= handoff point of chunk i+1) at step H[i]; starts at ST[i].
H = [L_IT - 1] + [L_IT - 1 + ((1023 - (L_IT - 1)) * i) // (NQ - 1)
     for i in range(1, NQ)]
ST = [h - (L_IT - 1) for h in H]
# chunk i>=1 passes its predecessor's end at iteration KSNAP[i] (7 or 8)
KSNAP = [None] + [H[i - 1] - ST[i] for i in range(1, NQ)]
KSNAP_LO = min(KSNAP[1:])
assert set(KSNAP[1:]) <= {KSNAP_LO, KSNAP_LO + 1}
# g DMA blocks per chain: graduated sizes so compute starts after a tiny
# first transfer instead of waiting for a multi-MB block.
BLENS = (1, 1, 2) + (4,) * 11     # sum = 48 = L_IT
BOFFS = [sum(BLENS[:i]) for i in range(len(BLENS))]
NEG = -10000.0

_CACHE = {}


def _build_bass():
    import concourse.bacc as bacc
    import concourse.mybir as mybir
    from concourse.tile import TileContext

    f32 = mybir.dt.float32
    bf16 = mybir.dt.bfloat16

    nc = bacc.Bacc()
    # g split per (block-size class, chain): gA/gB/gC hold iters [0,2), [2,4),
    # [4,48) as per-chain blocks of 1/2/4 iterations.
    gA = nc.dram_tensor("gA", [2, NCHAIN, 128, 1, MW * 32], bf16,
                        kind="ExternalInput")
    gB = nc.dram_tensor("gB", [1, NCHAIN, 128, 2, MW * 32], bf16,
                        kind="ExternalInput")
    gC = nc.dram_tensor("gC", [11, NCHAIN, 128, 4, MW * 32], bf16,
                        kind="ExternalInput")
    # Stationary operand: blockdiag(E^T, E^T) [128, 128]
    w_in = nc.dram_tensor("eaug", [128, 128], bf16, kind="ExternalInput")
    # whole-chain snapshots at the two handoff iterations (host picks per chunk)
    snapA = nc.dram_tensor("snapA", [NCHAIN, 128, MW * 32], bf16,
                           kind="ExternalOutput")
    snapB = nc.dram_tensor("snapB", [NCHAIN, 128, MW * 32], bf16,
                           kind="ExternalOutput")
    xfin = nc.dram_tensor("xfin", [NCHAIN, 128, MW * 32], bf16,
                          kind="ExternalOutput")
    xpen = nc.dram_tensor("xpen", [128, MW * 32], bf16, kind="ExternalOutput")

    NX = 8      # X state rotation slots per chain

    with TileContext(nc) as tc:
        with tc.tile_pool(name="const", bufs=1) as cpool, \
             tc.tile_pool(name="gp", bufs=4) as gpool, \
             tc.tile_pool(name="state", bufs=1) as xpool, \
             tc.tile_pool(name="ps", bufs=1, space="PSUM") as pspool:
            w_stage = cpool.tile([128, 128], bf16)
            nc.sync.dma_start(w_stage, w_in[:, :])
            w = cpool.tile([128, 128], bf16)
            # copy via DVE so matmuls depend only on the DVE semaphore
            nc.vector.tensor_copy(w, w_stage)
            xs, pss = [], []
            for ci in range(NCHAIN):
                rot = [xpool.tile([128, MW * 32], bf16, tag=f"x{ci}_{sl}",
                                  name=f"x{ci}_{sl}") for sl in range(NX)]
                xs.append(rot)
                pss.append(pspool.tile([128, MW * 32], f32, tag=f"ps{ci}",
                                       name=f"ps{ci}"))
            gts = [None] * NCHAIN
            for bi, (off, ln) in enumerate(zip(BOFFS, BLENS)):
                for ci in range(NCHAIN):
                    gt = gpool.tile([128, ln, MW * 32], bf16, tag=f"g{ci}_{ln}",
                                    name=f"gt{ci}_{bi}")
                    if ln == 1:
                        nc.sync.dma_start(gt, gA[off, ci])
                    elif ln == 2:
                        nc.sync.dma_start(gt, gB[0, ci])
                    else:
                        nc.sync.dma_start(gt, gC[(off - 4) // 4, ci])
                    gts[ci] = gt
                for t in range(ln):
                    k = off + t
                    for ci in range(NCHAIN):
                        gsl = gts[ci][:, t, :]
                        if k == 0:
                            nc.vector.tensor_copy(xs[ci][0], gsl)
                            continue
                        xprev = xs[ci][(k - 1) % NX]
                        xcur = xs[ci][k % NX]
                        nc.tensor.matmul(pss[ci], w, xprev, start=True, stop=True)
                        nc.vector.tensor_mul(xcur, gsl, pss[ci])
                        if k == KSNAP_LO:
                            nc.sync.dma_start(snapA[ci], xcur)
                        elif k == KSNAP_LO + 1:
                            nc.sync.dma_start(snapB[ci], xcur)
            for ci in range(NCHAIN):
                nc.sync.dma_start(xfin[ci], xs[ci][(L_IT - 1) % NX])
            nc.sync.dma_start(xpen[:, :], xs[NCHAIN - 1][(L_IT - 2) % NX])
    nc.finalize()
    return nc


def _numpy_ref(feats, masks, transitions):
    # Exact log-domain fallback (only used if masks are not all ones or the
    # fast path's safety checks trip).
    alpha = feats[:, 0].astype(np.float64)
    tr = transitions.astype(np.float64)
    for i in range(1, feats.shape[1]):
        sc = alpha[:, None, :] + tr[None] + feats[:, i, :, None].astype(np.float64)
        m = sc.max(axis=2, keepdims=True)
        new = (m[:, :, 0] + np.log(np.exp(sc - m).sum(axis=2)))
        mask = masks[:, i, None].astype(np.float64)
        alpha = new * mask + alpha * (1.0 - mask)
    return alpha.astype(np.float32)


def _estimate_F(feats, E):
    """Mean per-step log-growth of sum(X), sampled over a few batches in f64."""
    idx = np.linspace(0, feats.shape[0] - 1, 16).astype(int)
    Et = E.T.astype(np.float64)
    X = np.exp(feats[idx, 0].astype(np.float64))
    X /= X.sum(1, keepdims=True)
    cum = np.zeros(len(idx))
    cums = [cum.copy()]
    for s in range(1, feats.shape[1]):
        X = np.exp(feats[idx, s].astype(np.float64)) * (X @ Et)
        sm = X.sum(1)
        X /= sm[:, None]
        cum = cum + np.log(sm)
        cums.append(cum.copy())
    cums = np.stack(cums)
    F = float(cum.mean()) / (feats.shape[1] - 1)
    drift = cums - F * np.arange(cums.shape[0])[:, None]
    return F, float(np.abs(drift).max())


def _unpack(X):
    # [p=64h+j, m] -> [b_l = 32h+m, j]
    return X.reshape(2, T, 32).transpose(0, 2, 1).reshape(BL, T)


def kernel(feats, masks, transitions):
    feats = np.asarray(feats, dtype=np.float32)
    masks = np.asarray(masks, dtype=np.float32)
    transitions = np.asarray(transitions, dtype=np.float32)
    if not np.all(masks == 1.0):
        return _numpy_ref(feats, masks, transitions)

    from concourse import bass_utils

    if "nc" not in _CACHE:
        _CACHE["nc"] = _build_bass()
    nc = _CACHE["nc"]

    E = np.exp(transitions)                      # [j,k]; row/col 0 -> 0
    F, wander = _estimate_F(feats, E)
    if not (wander < 40.0):
        return _numpy_ref(feats, masks, transitions)

    Wmat = np.zeros((128, 128), np.float32)
    Wmat[:64, :64] = E.T
    Wmat[64:, 64:] = E.T
    import ml_dtypes
    g = np.exp(feats - F)
    # packed per core: G[core, s, p=64h+j, m] = g[b=core*64+32h+m, s, j]
    G = g.reshape(NC, 2, 32, S, T).transpose(0, 3, 1, 4, 2).reshape(NC, S, 128, 32)
    idx = (np.asarray(ST)[None, :] + np.arange(L_IT)[:, None])   # [L_IT, NQ]
    g_hw = G[:, idx]                             # [NC, L_IT, NQ, 128, 32]
    g_hw = g_hw.reshape(NC, L_IT, NCHAIN, MW, 128, 32).transpose(0, 1, 2, 4, 3, 5)
    g_hw = g_hw.reshape(NC, L_IT, NCHAIN, 128, MW * 32)
    g_hw = np.ascontiguousarray(g_hw, dtype=np.float32).astype(ml_dtypes.bfloat16)

    def blkview(c, off, ln):
        # [ln, NCHAIN, 128, 256] -> [NCHAIN, 128, ln, 256]
        return np.ascontiguousarray(
            g_hw[c, off:off + ln].transpose(1, 2, 0, 3))

    w16 = Wmat.astype(ml_dtypes.bfloat16)
    in_maps = []
    for c in range(NC):
        ga = np.stack([blkview(c, 0, 1), blkview(c, 1, 1)])
        gb = np.stack([blkview(c, 2, 2)])
        gc = np.stack([blkview(c, 4 + 4 * i, 4) for i in range(11)])
        in_maps.append({"gA": ga, "gB": gb, "gC": gc, "eaug": w16})
    trace = bool(os.environ.get("CRF_TRACE"))
    res = bass_utils.run_bass_kernel_spmd(
        nc, in_maps, core_ids=list(range(NC)), trace=trace)
    _CACHE["last_res"] = res

    alpha = np.empty((B, T), np.float32)
    ok = True
    for c in range(NC):
        r = res.results[c]
        snA = r["snapA"].astype(np.float64)      # [NCHAIN, 128, 256]
        snB = r["snapB"].astype(np.float64)
        fin = r["xfin"].astype(np.float64)
        pen = r["xpen"].astype(np.float64)

        def chunk_slice(arr, i):
            return _unpack(arr[i // MW][:, 32 * (i % MW):32 * (i % MW) + 32])

        lnr = np.zeros(BL)
        for i in range(1, NQ):
            sn = snA if KSNAP[i] == KSNAP_LO else snB
            snap_i = chunk_slice(sn, i)
            fin_prev = chunk_slice(fin, i - 1)
            rr = snap_i.sum(1) / np.maximum(fin_prev.sum(1), 1e-300)
            if not np.all(rr > 0):
                ok = False
            lnr += np.log(np.maximum(rr, 1e-300))
        zfin = chunk_slice(fin, NQ - 1)
        zpen = _unpack(pen[:, 32 * (MW - 1):])
        a = np.log(np.maximum(zfin, 1e-300)) + S * F - lnr[:, None]
        lsl = np.log(np.maximum(zpen.sum(1), 1e-300)) - lnr
        a[:, 0] = (feats[c * BL:(c + 1) * BL, S - 1, 0] + NEG
                   + lsl + (S - 1) * F)
        alpha[c * BL:(c + 1) * BL] = a.astype(np.float32)
    if not ok or not np.all(np.isfinite(alpha)):
        return _numpy_ref(feats, masks, transitions)
    return alpha
